# revision 24
# baseline (speedup 1.0000x reference)
"""DeltaNet fused kernel for 8 TRN2 NeuronCores (Bass/Tile), fp8-hybrid v2.

Math (reference, with W_fast_w == 0 so v_bar == W_fast_b):
    s  = x @ W_slow_w.T + W_slow_b            [B, 3073]
    k  = s[:, :1024]; v = s[:, 1024:2048]; q = s[:, 2048:3072]
    lr = sigmoid(s[:, 3072])
    delta[o,h] = sum_b (lr*(v - wfb))[b,o] * sigmoid(k)[b,h] / B
    out = softmax(q) @ delta.T + wfb

Restructured to eliminate the v projection (v = x @ Wv.T + bv):
    g  = lr * sigmoid(k)                      [B, H]
    M  = x.T @ g                              [I, H]   (per-core partial)
    r  = sum_b g[b, :]                        [H]
    delta.T = (M.T @ Wv.T + r x (bv - wfb)) / B        [H, O]  (AllReduced)
    out = softmax(q) @ delta.T + wfb

Precision (validated in numpy simulation; end-to-end rel err 6.6e-3 vs 2e-2
tolerance): fp8-e4m3 DoubleRow for q / M / final matmuls and the first half
of the k contraction; bf16 for the rest of k, lr, and the delta.T matmul.
The AllReduce runs in fp8 (delta scaled by 4096), split into two [H, 512]
column-halves so the final matmul's first half can start after the first AR.

Schedule: q chunk 0 warms up the PE while the k operands stream in; the
whole rest of the q-phase plus the softmax row-sums run after the AR
trigger to hide the collective; the final matmul is split into per-AR-half
passes. lr / r / rowsum are computed as transposed [1, N] matmuls (cheap
N=512 streams instead of 384 N=1 matmuls) and moved cross-partition via
tiny DRAM round-trips.
"""

import os
import sys

for _p in ("/opt/trn_rl_repo", "/root/.axon_site/_ro/trn_rl_repo"):
    if os.path.isdir(_p) and _p not in sys.path:
        sys.path.append(_p)

import numpy as np
import ml_dtypes

BF16 = ml_dtypes.bfloat16
F8E4 = ml_dtypes.float8_e4m3     # TRN fp8e4: max normal +-240

N_CORES = 8
B_FULL = 16384
DIM = 1024          # dim_in == dim_out == dim_hidden
P = 128
NT = DIM // P       # 8 tiles along any 1024 dim

SX = 16.0           # x fp8 scale             (|x|max 5.4  -> 87)
SWQ = 512.0         # Wq fp8 scale            (|w|max .16  -> 80)
SWK = 512.0         # Wk fp8 scale (i < 512 half)
SKP = SX * SWK      # k psum scale (bf16 half pre-scaled to match)
SWL = 512.0         # wlr fp8 scale (i < 512 half)
SG = 128.0          # g fp8 scale             (g in (0,1)  -> <128)
SAR = 4096.0        # delta fp8 scale         (|delta|max .043 -> 176)
SHIFT = 3.0         # exp shift               (max q+bq 7.63 -> et < 103)


def _build_program(b_core: int, n_cores: int = N_CORES):
    """Build the SPMD Bass program (same program on every core)."""
    import concourse.bass as bass
    import concourse.mybir as mybir
    import concourse.tile as tile
    from concourse import bacc

    f32 = mybir.dt.float32
    bf16 = mybir.dt.bfloat16
    f8 = mybir.dt.float8e4
    AF = mybir.ActivationFunctionType
    ALU = mybir.AluOpType
    DR = mybir.MatmulPerfMode.DoubleRow

    nbt = b_core // P               # b-tiles per core (16)
    nbc = b_core // 512             # 512-wide b-chunks (4)
    nct = 512 // P                  # b-tiles per chunk (4)
    assert b_core % 1024 == 0

    nc = bacc.Bacc(
        "TRN2",
        target_bir_lowering=False,
        debug=False,
        num_devices=n_cores,
    )

    # ---- kernel I/O ----
    xT16_h = nc.dram_tensor("xT16", [512, b_core], bf16, kind="ExternalInput")
    xT8_h = nc.dram_tensor("xT8", [DIM, b_core], f8, kind="ExternalInput")
    xn8_h = nc.dram_tensor("xn8", [b_core, DIM], f8, kind="ExternalInput")
    wk8_h = nc.dram_tensor("wk8", [512, DIM], f8, kind="ExternalInput")
    wk16_h = nc.dram_tensor("wk16", [512, DIM], bf16, kind="ExternalInput")
    wq8_h = nc.dram_tensor("wq8", [DIM, DIM], f8, kind="ExternalInput")
    wv16_h = nc.dram_tensor("wv16", [DIM, DIM], bf16, kind="ExternalInput")
    wlr8_h = nc.dram_tensor("wlr8", [512], f8, kind="ExternalInput")
    wlr16_h = nc.dram_tensor("wlr16", [512], bf16, kind="ExternalInput")
    bk_h = nc.dram_tensor("bk", [DIM], f32, kind="ExternalInput")     # bk * SKP
    bqs_h = nc.dram_tensor("bqs", [DIM], f32, kind="ExternalInput")   # bq - SHIFT
    blr_h = nc.dram_tensor("blr", [1], f32, kind="ExternalInput")
    # (bv - wfb) * SAR / (b_total * SG): outer-product operand for the drain
    bvcp_h = nc.dram_tensor("bvcp", [DIM], f32, kind="ExternalInput")
    wfb_h = nc.dram_tensor("wfb", [DIM], f32, kind="ExternalInput")
    out_h = nc.dram_tensor("out", [b_core, DIM], bf16, kind="ExternalOutput")

    pd_drain_scale = SAR / (float(b_core * n_cores) * SX * SG)
    fin_recip_scale = 1.0 / SAR
    q_act_scale = 1.0 / (SX * SWQ)
    k_act_scale = 1.0 / SKP
    lr_act_scale = 1.0 / (SX * SWL)

    with tile.TileContext(nc) as tc:
        with (
            tc.tile_pool(name="persist", bufs=1) as persist,
            tc.tile_pool(name="psum", bufs=6, space="PSUM") as psum,
            tc.tile_pool(name="psmall", bufs=2, space="PSUM") as psmall,
            tc.tile_pool(name="tmp", bufs=4) as tmp,
            tc.tile_pool(name="ost", bufs=6) as ost,
            tc.tile_pool(name="arst", bufs=4) as arst,
            tc.tile_pool(name="dram", bufs=1, space="DRAM") as dram,
        ):
            # ---- persistent SBUF tensors ----
            wq8 = persist.tile([P, NT, DIM], f8, name="wq8")
            xT8a = persist.tile([P, NT, b_core // 2], f8, name="xT8a")
            xT8b = persist.tile([P, NT, b_core // 2], f8, name="xT8b")
            wk8 = persist.tile([P, 4, DIM], f8, name="wk8")
            wk16 = persist.tile([P, 4, DIM], bf16, name="wk16")
            wlr8 = persist.tile([P, 4, 1], f8, name="wlr8")
            wlr16 = persist.tile([P, 4, 1], bf16, name="wlr16")
            xT16 = persist.tile([P, 4, b_core], bf16, name="xT16")
            xn8 = persist.tile([P, nbt, DIM], f8, name="xn8")
            wv16 = persist.tile([P, NT, DIM], bf16, name="wv16")
            g8 = persist.tile([P, nbt, DIM], f8, name="g8")
            et8 = persist.tile([P, NT, b_core], f8, name="et8")
            mb = persist.tile([P, NT, DIM], bf16, name="mb")
            wn8a = persist.tile([P, NT, 512], f8, name="wn8a")
            wn8b = persist.tile([P, NT, 512], f8, name="wn8b")
            bk_b = persist.tile([P, DIM], f32, name="bk_b")
            wfb_b = persist.tile([P, DIM], f32, name="wfb_b")
            bvcp_b = persist.tile([P, DIM], f32, name="bvcp_b")
            bq_c = persist.tile([P, NT], f32, name="bq_c")
            blr_c = persist.tile([P, 1], f32, name="blr_c")
            lr_c = persist.tile([P, nbt], f32, name="lr_c")
            r_c = persist.tile([P, NT], f32, name="r_c")
            recip_c = persist.tile([P, nbt], f32, name="recip_c")
            lrT_sb = persist.tile([1, b_core], f32, name="lrT_sb")
            rT_sb = persist.tile([1, DIM], f32, name="rT_sb")
            prsT_sb = persist.tile([1, b_core], f32, name="prsT_sb")
            ones8 = persist.tile([P, 2, 16], f8, name="ones8")
            ones_row = persist.tile([1, P], f32, name="ones_row")

            # ---- DRAM: AllReduce bounce (column halves) + transpose scratch ----
            ar_inA = dram.tile([DIM, 512], f8, name="ar_inA")
            ar_inB = dram.tile([DIM, 512], f8, name="ar_inB")
            ar_outA = dram.tile([DIM, 512], f8, name="ar_outA", addr_space="Shared")
            ar_outB = dram.tile([DIM, 512], f8, name="ar_outB", addr_space="Shared")
            sc_lr = dram.tile([nbt, P], f32, name="sc_lr")
            sc_r = dram.tile([NT, P], f32, name="sc_r")
            sc_prs = dram.tile([nbt, P], f32, name="sc_prs")
            warm_in = dram.tile([P, 512], f8, name="warm_in")
            warm_out = dram.tile([P, 512], f8, name="warm_out", addr_space="Shared")

            nc.vector.memset(ones8[:], 1.0)
            nc.vector.memset(ones_row[:], 1.0)

            # warm up the collective stream: the first collective pays a
            # ~50us barrier/setup cost; burn it on a tiny AllReduce that
            # runs concurrently with the startup DMAs.
            wtmp = tmp.tile([P, 512], f8, tag="kv", name="wtmp")
            nc.vector.memset(wtmp[:], 0.0)
            nc.gpsimd.dma_start(out=warm_in[:, :], in_=wtmp[:])
            nc.gpsimd.collective_compute(
                "AllReduce",
                mybir.AluOpType.add,
                replica_groups=[list(range(n_cores))],
                ins=[warm_in[:, :]],
                outs=[warm_out[:, :]],
            )

            # ---- small DMAs (gpsimd queue) ----
            nc.gpsimd.dma_start(
                out=bq_c[:],
                in_=bass.AP(tensor=bqs_h, offset=0, ap=[[1, P], [P, NT]]),
            )
            nc.gpsimd.dma_start(
                out=blr_c[:],
                in_=bass.AP(tensor=blr_h, offset=0, ap=[[0, P], [1, 1]]),
            )
            for i in range(4):
                nc.gpsimd.dma_start(
                    out=wlr8[:, i, :],
                    in_=bass.AP(tensor=wlr8_h, offset=i * P, ap=[[1, P], [P, 1]]),
                )
                nc.gpsimd.dma_start(
                    out=wlr16[:, i, :],
                    in_=bass.AP(tensor=wlr16_h, offset=i * P, ap=[[1, P], [P, 1]]),
                )
            # bias broadcasts across partitions via K=1 ones-matmuls
            for bi, (bias_dst, bias_src) in enumerate(
                ((bk_b, bk_h), (wfb_b, wfb_h), (bvcp_b, bvcp_h))
            ):
                for c in range(2):
                    brow = tmp.tile([1, 512], f32, tag="kv", name=f"br{bi}_{c}")
                    nc.scalar.dma_start(
                        out=brow[:],
                        in_=bass.AP(tensor=bias_src, offset=c * 512,
                                    ap=[[0, 1], [1, 512]]),
                    )
                    pb = psum.tile([P, 512], f32, tag="ps", name=f"pb{bi}_{c}")
                    nc.tensor.matmul(
                        pb[:], ones_row[:, :], brow[:], start=True, stop=True
                    )
                    nc.vector.tensor_copy(bias_dst[:, c * 512:(c + 1) * 512], pb[:])

            # ---- bulk DMAs: sync queue = q/k weights + xT8; scalar = rest ----
            for i in range(NT):
                nc.sync.dma_start(out=wq8[:, i, :], in_=wq8_h[i * P:(i + 1) * P, :])
            for i in range(NT):
                nc.sync.dma_start(
                    out=xT8a[:, i, 0:512], in_=xT8_h[i * P:(i + 1) * P, 0:512]
                )
            for i in range(4):
                nc.sync.dma_start(out=wk8[:, i, :], in_=wk8_h[i * P:(i + 1) * P, :])
            for i in range(4):
                nc.sync.dma_start(out=wk16[:, i, :], in_=wk16_h[i * P:(i + 1) * P, :])
            if b_core // 2 > 512:
                for i in range(NT):
                    nc.sync.dma_start(
                        out=xT8a[:, i, 512:],
                        in_=xT8_h[i * P:(i + 1) * P, 512:b_core // 2],
                    )
            for i in range(NT):
                nc.sync.dma_start(
                    out=xT8b[:, i, :], in_=xT8_h[i * P:(i + 1) * P, b_core // 2:]
                )
            # scalar queue: xT16 b-chunked (earliest b first), then xn8, wv16
            for c in range(nbc):
                for i in range(4):
                    nc.scalar.dma_start(
                        out=xT16[:, i, c * 512:(c + 1) * 512],
                        in_=xT16_h[i * P:(i + 1) * P, c * 512:(c + 1) * 512],
                    )
            for t in range(nbt):
                nc.scalar.dma_start(out=xn8[:, t, :], in_=xn8_h[t * P:(t + 1) * P, :])
            for i in range(NT):
                nc.scalar.dma_start(out=wv16[:, i, :], in_=wv16_h[i * P:(i + 1) * P, :])

            def xt8_lhs(t, j2):
                """fp8 xT lhsT pair slice for global b-tile t, i-pair j2."""
                src = xT8a if t < nbt // 2 else xT8b
                tc_ = t % (nbt // 2)
                return src[:, 2 * j2:2 * j2 + 2, tc_ * P:(tc_ + 1) * P]

            def emit_q(chunks):
                """et8 = exp(qT + bq - SHIFT), transposed layout [h, b]. fp8 DR."""
                for bc in chunks:
                    src = xT8a if bc < nbc // 2 else xT8b
                    lo = (bc % (nbc // 2)) * 512
                    for hb in range(NT):
                        pq = psum.tile([P, 512], f32, tag="ps", name=f"pq{bc}_{hb}")
                        for j in range(NT // 2):
                            nc.tensor.matmul(
                                pq[:],
                                wq8[:, 2 * j:2 * j + 2, hb * P:(hb + 1) * P],
                                src[:, 2 * j:2 * j + 2, lo:lo + 512],
                                start=(j == 0), stop=(j == NT // 2 - 1),
                                perf_mode=DR,
                            )
                        nc.scalar.activation(
                            et8[:, hb, bc * 512:(bc + 1) * 512], pq[:], AF.Exp,
                            bias=bq_c[:, hb:hb + 1], scale=q_act_scale,
                        )

            def emit_lrT(bc):
                """lr_c[p, 4bc+j] = sigmoid(x @ wlr + blr) * SG for one
                512-col chunk, via a transposed [1, 512] matmul + bounce."""
                src8 = xT8a if bc < nbc // 2 else xT8b
                lo = (bc % (nbc // 2)) * 512
                pl = psmall.tile([1, 512], f32, tag="pl", name=f"plr{bc}")
                for i in range(4):
                    nc.tensor.matmul(
                        pl[:],
                        wlr8[:, i, 0:1],
                        src8[:, i, lo:lo + 512],
                        start=(i == 0), stop=False,
                    )
                for i in range(4):
                    nc.tensor.matmul(
                        pl[:],
                        wlr16[:, i, 0:1],
                        xT16[:, i, bc * 512:(bc + 1) * 512],
                        start=False, stop=(i == 3),
                    )
                nc.scalar.activation(
                    lrT_sb[0:1, bc * 512:(bc + 1) * 512], pl[:], AF.Sigmoid,
                    bias=blr_c[0:1, 0:1], scale=lr_act_scale,
                )
                nc.gpsimd.dma_start(
                    out=sc_lr[nct * bc:nct * (bc + 1), :],
                    in_=lrT_sb[0:1, bc * 512:(bc + 1) * 512],
                )
                nc.gpsimd.dma_start(
                    out=lr_c[:, nct * bc:nct * (bc + 1)],
                    in_=sc_lr[nct * bc:nct * (bc + 1), :].rearrange("a b -> b a"),
                )
                nc.vector.tensor_scalar_mul(
                    lr_c[:, nct * bc:nct * (bc + 1)],
                    lr_c[:, nct * bc:nct * (bc + 1)], SG,
                )

            def emit_k(tiles):
                """g8 = lr * sigmoid(k) * SG, natural layout [b, h].
                Contraction split: i<512 fp8-DR, i>=512 bf16."""
                for t in tiles:
                    for c in range(2):
                        pk = psum.tile([P, 512], f32, tag="ps", name=f"pk{t}_{c}")
                        for j2 in range(2):
                            nc.tensor.matmul(
                                pk[:],
                                xt8_lhs(t, j2),
                                wk8[:, 2 * j2:2 * j2 + 2, c * 512:(c + 1) * 512],
                                start=(j2 == 0), stop=False,
                                perf_mode=DR,
                            )
                        for i in range(4):
                            nc.tensor.matmul(
                                pk[:],
                                xT16[:, i, t * P:(t + 1) * P],
                                wk16[:, i, c * 512:(c + 1) * 512],
                                start=False, stop=(i == 3),
                            )
                        ktmp = tmp.tile([P, 512], f32, tag="kv", name=f"kt{t}_{c}")
                        nc.vector.tensor_add(
                            ktmp[:], pk[:], bk_b[:, c * 512:(c + 1) * 512]
                        )
                        sgk = tmp.tile([P, 512], bf16, tag="sg", name=f"sg{t}_{c}")
                        nc.scalar.activation(sgk[:], ktmp[:], AF.Sigmoid,
                                             scale=k_act_scale)
                        nc.scalar.activation(
                            g8[:, t, c * 512:(c + 1) * 512], sgk[:], AF.Copy,
                            scale=lr_c[:, t:t + 1],
                        )

            def emit_m():
                """mb = x.T @ g (per-core partial), [i, h] layout, fp8 DR;
                then rT = ones.T @ g via [1, 512] matmuls + bounce."""
                for hc in range(2):
                    for ib in range(NT):
                        pm = psum.tile([P, 512], f32, tag="ps", name=f"pm{hc}_{ib}")
                        for bp in range(nbt // 2):
                            nc.tensor.matmul(
                                pm[:],
                                xn8[:, 2 * bp:2 * bp + 2, ib * P:(ib + 1) * P],
                                g8[:, 2 * bp:2 * bp + 2, hc * 512:(hc + 1) * 512],
                                start=(bp == 0), stop=(bp == nbt // 2 - 1),
                                perf_mode=DR,
                            )
                        nc.vector.tensor_copy(
                            mb[:, ib, hc * 512:(hc + 1) * 512], pm[:]
                        )
                for hc in range(2):
                    pr = psmall.tile([1, 512], f32, tag="pl", name=f"pr{hc}")
                    for bp in range(nbt // 2):
                        nc.tensor.matmul(
                            pr[:],
                            ones8[:, 0:2, 0:1],
                            g8[:, 2 * bp:2 * bp + 2, hc * 512:(hc + 1) * 512],
                            start=(bp == 0), stop=(bp == nbt // 2 - 1),
                            perf_mode=DR,
                        )
                    nc.vector.tensor_copy(
                        rT_sb[0:1, hc * 512:(hc + 1) * 512], pr[:]
                    )
                    nc.gpsimd.dma_start(
                        out=sc_r[nct * hc:nct * (hc + 1), :],
                        in_=rT_sb[0:1, hc * 512:(hc + 1) * 512],
                    )
                nc.gpsimd.dma_start(
                    out=r_c[:, :], in_=sc_r[:, :].rearrange("a b -> b a")
                )

            def emit_pd(oc):
                """delta.T partial [:, oc half] = mb.T @ wv + r x bvc,
                drained fp8 to one AR column-half."""
                for hb in range(NT):
                    pd = psum.tile([P, 512], f32, tag="ps", name=f"pd{hb}_{oc}")
                    for i in range(NT):
                        nc.tensor.matmul(
                            pd[:],
                            mb[:, i, hb * P:(hb + 1) * P],
                            wv16[:, i, oc * 512:(oc + 1) * 512],
                            start=(i == 0), stop=(i == NT - 1),
                        )
                    pt = tmp.tile([P, 512], f32, tag="kv", name=f"pt{hb}_{oc}")
                    nc.scalar.activation(
                        pt[:], pd[:], AF.Copy, scale=pd_drain_scale
                    )
                    dst = arst.tile([P, 512], f8, tag="ar", name=f"ds{hb}_{oc}")
                    nc.vector.scalar_tensor_tensor(
                        dst[:],
                        bvcp_b[:, oc * 512:(oc + 1) * 512],
                        r_c[:, hb:hb + 1],
                        pt[:],
                        op0=ALU.mult,
                        op1=ALU.add,
                    )
                    ar_dst = ar_inA if oc == 0 else ar_inB
                    eng = nc.sync if oc == 0 else nc.scalar
                    eng.dma_start(
                        out=ar_dst[hb * P:(hb + 1) * P, :], in_=dst[:]
                    )

            def emit_prsT(bc):
                """prsT[b] = sum_h et8[h, b] for one 512-col chunk."""
                pp = psmall.tile([1, 512], f32, tag="pl", name=f"pp{bc}")
                for j in range(NT // 2):
                    nc.tensor.matmul(
                        pp[:],
                        ones8[:, 0:2, 0:1],
                        et8[:, 2 * j:2 * j + 2, bc * 512:(bc + 1) * 512],
                        start=(j == 0), stop=(j == NT // 2 - 1),
                        perf_mode=DR,
                    )
                nc.vector.tensor_copy(prsT_sb[0:1, bc * 512:(bc + 1) * 512], pp[:])
                nc.sync.dma_start(
                    out=sc_prs[nct * bc:nct * (bc + 1), :],
                    in_=prsT_sb[0:1, bc * 512:(bc + 1) * 512],
                )

            def emit_recip():
                nc.sync.dma_start(
                    out=recip_c[:, :], in_=sc_prs[:, :].rearrange("a b -> b a")
                )
                nc.vector.reciprocal(recip_c[:], recip_c[:])
                nc.vector.tensor_scalar_mul(recip_c[:], recip_c[:], fin_recip_scale)

            def emit_fin(oc, wn):
                """out[:, oc half] = (et8.T @ wn) * recip + wfb, fp8 DR."""
                for t in range(nbt):
                    po = psum.tile([P, 512], f32, tag="ps", name=f"po{t}_{oc}")
                    for j in range(NT // 2):
                        nc.tensor.matmul(
                            po[:],
                            et8[:, 2 * j:2 * j + 2, t * P:(t + 1) * P],
                            wn[:, 2 * j:2 * j + 2, :],
                            start=(j == 0), stop=(j == NT // 2 - 1),
                            perf_mode=DR,
                        )
                    o_st = ost.tile([P, 512], bf16, tag="os", name=f"os{t}_{oc}")
                    nc.vector.scalar_tensor_tensor(
                        o_st[:],
                        po[:],
                        recip_c[:, t:t + 1],
                        wfb_b[:, oc * 512:(oc + 1) * 512],
                        op0=ALU.mult,
                        op1=ALU.add,
                    )
                    eng = nc.sync if oc == 0 else nc.scalar
                    eng.dma_start(
                        out=out_h[t * P:(t + 1) * P, oc * 512:(oc + 1) * 512],
                        in_=o_st[:],
                    )

            # ---- schedule ----
            emit_q([0])           # q chunk 0 warms up the PE
            emit_lrT(0)
            for bc in range(1, nbc):
                emit_k(range(nct * (bc - 1), nct * bc))
                emit_lrT(bc)
            emit_k(range(nct * (nbc - 1), nct * nbc))
            emit_m()
            emit_pd(0)
            nc.gpsimd.collective_compute(
                "AllReduce",
                mybir.AluOpType.add,
                replica_groups=[list(range(n_cores))],
                ins=[ar_inA[:, :]],
                outs=[ar_outA[:, :]],
            )
            emit_pd(1)
            nc.gpsimd.collective_compute(
                "AllReduce",
                mybir.AluOpType.add,
                replica_groups=[list(range(n_cores))],
                ins=[ar_inB[:, :]],
                outs=[ar_outB[:, :]],
            )
            for hb in range(NT):
                nc.scalar.dma_start(
                    out=wn8a[:, hb, :], in_=ar_outA[hb * P:(hb + 1) * P, :]
                )
            for hb in range(NT):
                nc.scalar.dma_start(
                    out=wn8b[:, hb, :], in_=ar_outB[hb * P:(hb + 1) * P, :]
                )
            emit_prsT(0)
            for bc in range(1, nbc):
                emit_q([bc])
                emit_prsT(bc)
            emit_recip()
            emit_fin(0, wn8a)
            emit_fin(1, wn8b)

    nc.compile()
    return nc


def _host_prep(x, W_slow_w, W_slow_b, W_fast_b, b_core, n_cores):
    """Shard + pre-transpose + cast inputs; returns per-core input maps."""
    Wk = W_slow_w[:DIM]
    Wv = W_slow_w[DIM:2 * DIM]
    Wq = W_slow_w[2 * DIM:3 * DIM]
    wlr = W_slow_w[3 * DIM]

    WkT = np.ascontiguousarray(Wk.T)
    wk8 = np.clip(WkT[:512, :] * SWK, -240.0, 240.0).astype(F8E4)
    wk16 = (WkT[512:, :] * SKP).astype(BF16)
    wv16 = np.ascontiguousarray(Wv.T).astype(BF16)
    wq8 = np.clip(np.ascontiguousarray(Wq.T) * SWQ, -240.0, 240.0).astype(F8E4)
    wlr8 = np.clip(wlr[:512] * SWL, -240.0, 240.0).astype(F8E4)
    wlr16 = (wlr[512:] * (SX * SWL)).astype(BF16)

    bk = (W_slow_b[:DIM] * SKP).astype(np.float32)
    b_total = float(b_core * n_cores)
    bvcp = ((W_slow_b[DIM:2 * DIM] - W_fast_b) * (SAR / (b_total * SG))).astype(
        np.float32
    )
    bqs = (W_slow_b[2 * DIM:3 * DIM] - SHIFT).astype(np.float32)
    blr = np.ascontiguousarray(W_slow_b[3 * DIM:3 * DIM + 1]).astype(np.float32)
    wfb = np.ascontiguousarray(W_fast_b).astype(np.float32)

    in_maps = []
    for c in range(n_cores):
        xs = x[c * b_core:(c + 1) * b_core, :]
        xT = np.ascontiguousarray(xs.T)
        xT16 = np.ascontiguousarray(xT[512:]).astype(BF16)
        xT8 = np.clip(xT * SX, -240.0, 240.0).astype(F8E4)
        xn8 = np.clip(xs * SX, -240.0, 240.0).astype(F8E4)
        in_maps.append({
            "xT16": xT16, "xT8": xT8, "xn8": np.ascontiguousarray(xn8),
            "wk8": wk8, "wk16": wk16, "wq8": wq8, "wv16": wv16,
            "wlr8": wlr8, "wlr16": wlr16,
            "bk": bk, "bqs": bqs, "blr": blr, "bvcp": bvcp, "wfb": wfb,
        })
    return in_maps


_PROGRAM_CACHE = {}


def _get_program(b_core, n_cores=N_CORES):
    key = (b_core, n_cores)
    if key not in _PROGRAM_CACHE:
        _PROGRAM_CACHE[key] = _build_program(b_core, n_cores)
    return _PROGRAM_CACHE[key]


def _run_device(x, W_slow_w, W_slow_b, W_fast_b, trace=False):
    from concourse.bass_utils import run_bass_kernel_spmd

    b_core = x.shape[0] // N_CORES
    nc = _get_program(b_core)
    in_maps = _host_prep(x, W_slow_w, W_slow_b, W_fast_b, b_core, N_CORES)
    res = run_bass_kernel_spmd(nc, in_maps, list(range(N_CORES)), trace=trace)
    out = np.concatenate([res.results[c]["out"] for c in range(N_CORES)], axis=0)
    return out.astype(np.float32), res


def _reference_numpy(x, W_slow_w, W_slow_b, W_fast_w, W_fast_b):
    """Exact fallback (only used if W_fast_w != 0, which the spec never produces)."""
    x = x.astype(np.float64)
    s = x @ W_slow_w.astype(np.float64).T + W_slow_b.astype(np.float64)
    k = s[:, :DIM]
    v = s[:, DIM:2 * DIM]
    q = s[:, 2 * DIM:3 * DIM]
    lr = 1.0 / (1.0 + np.exp(-s[:, -1:]))
    ek = np.exp(k - k.max(axis=1, keepdims=True))
    ak = ek / ek.sum(axis=1, keepdims=True)
    v_bar = ak @ W_fast_w.astype(np.float64).T + W_fast_b.astype(np.float64)
    sigk = 1.0 / (1.0 + np.exp(-k))
    delta = (lr * (v - v_bar)).T @ sigk / x.shape[0]
    w_new = W_fast_w.astype(np.float64) + delta
    eq = np.exp(q - q.max(axis=1, keepdims=True))
    aq = eq / eq.sum(axis=1, keepdims=True)
    return (aq @ w_new.T + W_fast_b.astype(np.float64)).astype(np.float32)


def kernel(x, W_slow_w, W_slow_b, W_fast_w, W_fast_b):
    x = np.asarray(x)
    W_slow_w = np.asarray(W_slow_w)
    W_slow_b = np.asarray(W_slow_b)
    W_fast_w = np.asarray(W_fast_w)
    W_fast_b = np.asarray(W_fast_b)
    if np.any(W_fast_w):
        # Spec guarantees W_fast_w == 0; exact fallback for generality.
        return _reference_numpy(x, W_slow_w, W_slow_b, W_fast_w, W_fast_b)
    out, _ = _run_device(x, W_slow_w, W_slow_b, W_fast_b, trace=False)
    return out
